# revision 4
# baseline (speedup 1.0000x reference)
"""DGCNN_cls on Trainium2: data-parallel over the batch dim across 8 NeuronCores.

Sharding: B=16 point clouds -> 8 cores x 2 samples (pure data parallel, per
the sharding hint); the small weights are replicated. The training-mode
BatchNorms couple samples across cores, so per-core partial sums/sumsq are
combined with jax.lax.psum inside a pmap over the 8 cores; everything else
(kNN, gather, edge convs, pools, head) is computed per-core on-device.

kernel(**inputs) takes the FULL unsharded inputs and returns the FULL output
tuple (out[16,40], node1[16,3,256], node2[16,3,64]) matching reference().
"""
import numpy as np
import jax
import jax.numpy as jnp
from functools import partial

EPS = 1e-5
SLOPE = 0.2
AXIS = "cores"
NCORES = 8

_cache = {}


def _lrelu(x):
    return jnp.where(x >= 0, x, SLOPE * x)


def _bn_dist(x, axes):
    """BatchNorm over `axes` (which always include the batch axis 0), with the
    batch dim sharded across cores: combine moments with psum."""
    n_loc = 1
    for a in axes:
        n_loc *= x.shape[a]
    s1 = jax.lax.psum(jnp.sum(x, axis=axes, keepdims=True), AXIS)
    s2 = jax.lax.psum(jnp.sum(x * x, axis=axes, keepdims=True), AXIS)
    n = n_loc * NCORES
    m = s1 / n
    v = s2 / n - m * m
    return (x - m) * jax.lax.rsqrt(v + EPS)


def _knn(x, k):
    inner = -2.0 * jnp.einsum("bcn,bcm->bnm", x, x)
    xx = jnp.sum(x * x, axis=1)
    d = -xx[:, :, None] - inner - xx[:, None, :]
    return jax.lax.top_k(d, k)[1]


def _graph_feature(x, k):
    idx = _knn(x, k)
    xt = jnp.transpose(x, (0, 2, 1))
    # Chunked gathers: keep each indirect-load instruction's element count
    # under the 16-bit DMA-semaphore limit of the compiler (65535).
    B, N = x.shape[0], x.shape[2]
    nch = max(1, (N * k) // 16384)
    feats = []
    for b in range(B):
        rows = []
        step = N // nch
        for c in range(nch):
            rows.append(xt[b][idx[b, c * step:(c + 1) * step]])
        feats.append(jnp.concatenate(rows, 0) if nch > 1 else rows[0])
    feat = jnp.stack(feats)
    center = xt[:, :, None, :]
    out = jnp.concatenate([feat - center, jnp.broadcast_to(center, feat.shape)], axis=-1)
    return jnp.transpose(out, (0, 3, 1, 2))


def _conv2(w, x):
    return jnp.einsum("oc,bcnk->bonk", w, x)


def _conv1(w, x):
    return jnp.einsum("oc,bcn->bon", w, x)


def _pool(xyz, feature, wp, bp, kp):
    proj = _conv1(wp, feature) + bp[None, :, None]
    vector = jnp.max(proj, axis=-1, keepdims=True)
    scores = jax.nn.sigmoid(jnp.sum(feature * vector, axis=1))
    values, idx = jax.lax.top_k(scores, kp)
    node_static = jnp.take_along_axis(xyz, idx[:, None, :], axis=2)
    node_feature = jnp.take_along_axis(feature, idx[:, None, :], axis=2)
    v = values[:, None, :]
    return node_static * v, node_feature * v


def _stage1a(x, w1):
    h = _lrelu(_bn_dist(_conv2(w1, _graph_feature(x, 20)), (0, 2, 3)))
    return jnp.max(h, axis=-1)


def _stage1b(x1, w2):
    h = _lrelu(_bn_dist(_conv2(w2, _graph_feature(x1, 20)), (0, 2, 3)))
    return jnp.max(h, axis=-1)


def _stage_rest(x, x1, x2, w2m, w3, w4, w4m, w5, w5m, wp1, bp1, wp2, bp2,
                wl1, wl2, bl2, wl3, bl3):
    k = 20
    xyz = x
    xt1_ = jnp.concatenate([x1, x2], axis=1)
    xt1 = _lrelu(_bn_dist(_conv1(w2m, xt1_), (0, 2)))
    node1, nf1 = _pool(xyz, xt1_, wp1, bp1, 256)
    h = _lrelu(_bn_dist(_conv2(w3, _graph_feature(nf1, k // 2)), (0, 2, 3)))
    x3 = jnp.max(h, axis=-1)
    h = _lrelu(_bn_dist(_conv2(w4, _graph_feature(x3, k // 2)), (0, 2, 3)))
    x4 = jnp.max(h, axis=-1)
    xt2_ = jnp.concatenate([x3, x4], axis=1)
    xt2 = _lrelu(_bn_dist(_conv1(w4m, xt2_), (0, 2)))
    node2, nf2 = _pool(node1, xt2_, wp2, bp2, 64)
    h = _lrelu(_bn_dist(_conv2(w5, _graph_feature(nf2, k // 4)), (0, 2, 3)))
    x5 = jnp.max(h, axis=-1)
    xt3 = _lrelu(_bn_dist(_conv1(w5m, x5), (0, 2)))
    g = jnp.concatenate([jnp.max(xt1, -1), jnp.max(xt2, -1), jnp.max(xt3, -1)], axis=1)
    h = _lrelu(_bn_dist(g @ wl1.T, (0,)))
    h = _lrelu(_bn_dist(h @ wl2.T + bl2, (0,)))
    out = h @ wl3.T + bl3
    return out, node1, node2


def _get_pmapped():
    if "fn" not in _cache:
        dev = jax.devices()[:NCORES]
        _cache["f1"] = jax.pmap(_stage1a, axis_name=AXIS,
                                in_axes=(0, None), devices=dev)
        _cache["f2"] = jax.pmap(_stage1b, axis_name=AXIS,
                                in_axes=(0, None), devices=dev)
        _cache["f3"] = jax.pmap(_stage_rest, axis_name=AXIS,
                                in_axes=(0, 0, 0) + (None,) * 15, devices=dev)
        _cache["fn"] = 1
    return _cache["f1"], _cache["f2"], _cache["f3"]


def kernel(x, w1, w2, w2m, w3, w4, w4m, w5, w5m, wp1, bp1, wp2, bp2,
           wl1, wl2, bl2, wl3, bl3):
    x = np.asarray(x, dtype=np.float32)
    xs = x.reshape(NCORES, x.shape[0] // NCORES, 3, x.shape[2])
    f1, f2, f3 = _get_pmapped()
    xs = jnp.asarray(xs)
    x1 = f1(xs, w1)
    x2 = f2(x1, w2)
    out, node1, node2 = f3(xs, x1, x2, w2m, w3, w4, w4m, w5, w5m,
                           wp1, bp1, wp2, bp2, wl1, wl2, bl2, wl3, bl3)
    out = np.asarray(out).reshape(16, -1)
    node1 = np.asarray(node1).reshape(16, 3, 256)
    node2 = np.asarray(node2).reshape(16, 3, 64)
    return (out.astype(np.float32), node1.astype(np.float32),
            node2.astype(np.float32))
